# revision 4
# baseline (speedup 1.0000x reference)
import sys

import numpy as np

if "/opt/trn_rl_repo" not in sys.path:
    sys.path.insert(0, "/opt/trn_rl_repo")

import ml_dtypes

BF = ml_dtypes.bfloat16

B = 4
H = 128
F_OUT = 3
NBLK = 3
N_FULL = 10000
P_CORES = 8
F_IN = 963


def _tiles(total, t=128):
    out = []
    while total > 0:
        out.append(min(t, total))
        total -= t
    return out


def _kbatches(rows, maxb=4):
    full = rows // 128
    rem = rows % 128
    bs = []
    k = 0
    while k < full:
        n = min(maxb, full - k)
        bs.append((k, n, 128))
        k += n
    if rem:
        bs.append((full, 1, rem))
    return bs


def build_nc(N, P, F_in):
    """SPMD Bass program (identical on all cores).

    adj rows are sharded (NL per core). Row tiles are split into two halves
    (A = first 5 m-tiles, B = rest); each half's transformed features are
    all-gathered separately so the collective for one half hides under the
    adjacency matmuls of the other. adjT is host-permuted so the gathered
    [A-of-all-cores; B-of-all-cores] row order matches its k dimension.
    """
    from concourse import bacc, tile, mybir

    f32 = mybir.dt.float32
    bf16 = mybir.dt.bfloat16
    Relu = mybir.ActivationFunctionType.Relu
    Copy = mybir.ActivationFunctionType.Copy
    add = mybir.AluOpType.add

    NL = N // P
    MT = _tiles(NL)
    NMT = len(MT)
    m_offs = [sum(MT[:i]) for i in range(NMT)]
    assert NMT >= 2
    NMT_A = (NMT + 1) // 2
    A_mts = list(range(NMT_A))
    B_mts = list(range(NMT_A, NMT))
    NA = sum(MT[:NMT_A])
    NBr = NL - NA
    PA, PB = P * NA, P * NBr
    assert PA % 128 == 0
    KA = PA // 128
    KBt = (PB + 127) // 128
    NKT = KA + KBt
    bA = _kbatches(PA)
    bB = _kbatches(PB)
    KF = (F_in + 127) // 128
    MAXW = max(NA, NBr)
    out_chunks = []
    c = 0
    while c < NL:
        out_chunks.append((c, min(c + 512, NL)))
        c += 512

    nc = bacc.Bacc(trn_type="TRN2", target_bir_lowering=False, num_devices=P)

    x0T = nc.dram_tensor("x0T", [B, KF, 128, NL], bf16, kind="ExternalInput")
    adjT = nc.dram_tensor("adjT", [NKT, 128, NL], bf16, kind="ExternalInput")
    w_in = nc.dram_tensor("w_in", [KF, 128, 2 * H], bf16, kind="ExternalInput")
    w_res = nc.dram_tensor("w_res", [2 * NBLK, 128, 2 * H], bf16, kind="ExternalInput")
    w_out = nc.dram_tensor("w_out", [128, 2 * F_OUT], bf16, kind="ExternalInput")
    x_res_out = nc.dram_tensor("x_res_out", [B, NL, H], f32, kind="ExternalOutput")
    x_out_t = nc.dram_tensor("x_out_t", [B * F_OUT, NL], f32, kind="ExternalOutput")

    replica_groups = [list(range(P))]

    with tile.TileContext(nc) as tc:
        with (
            tc.tile_pool(name="wp", bufs=1) as wp,
            tc.tile_pool(name="sbp", bufs=1) as sbp,
            tc.tile_pool(name="stp", bufs=1) as stp,
            tc.tile_pool(name="pp", bufs=1, space="PSUM") as pp,
            tc.tile_pool(name="dp", bufs=2, space="DRAM") as dp,
        ):
            w_in_t = []
            for kf in range(KF):
                wt = wp.tile([128, 2 * H], bf16, tag=f"w_in{kf}", name=f"w_in{kf}")
                nc.sync.dma_start(wt[:, :], w_in[kf, :, :])
                w_in_t.append(wt)
            w_res_t = []
            for i in range(2 * NBLK):
                wt = wp.tile([128, 2 * H], bf16, tag=f"w_res{i}", name=f"w_res{i}")
                nc.sync.dma_start(wt[:, :], w_res[i, :, :])
                w_res_t.append(wt)
            w_out_t = wp.tile([128, 2 * F_OUT], bf16, tag="w_out", name="w_out")
            nc.sync.dma_start(w_out_t[:, :], w_out[:, :])
            xlwoT_sb = sbp.tile([F_OUT * B, NL], f32, tag="xlwoT", name="xlwoT_sb")

            xT_cur = {}
            xlw_cur = {}
            xres_cur = {}
            st = {"pk": 0}

            def pk_banks(n):
                c = st["pk"]
                st["pk"] = c + n
                return [f"pk{(c + i) % 6}" for i in range(n)]

            def emit_ag(bounce, gath):
                nc.gpsimd.collective_compute(
                    "AllGather",
                    mybir.AluOpType.bypass,
                    replica_groups=replica_groups,
                    ins=[bounce.opt()],
                    outs=[gath.opt()],
                )

            def emit_feature_h(g_next, mt, bounce, roff):
                m, moff = MT[mt], m_offs[mt]
                wt = w_res_t[g_next - 1]
                xlw_new = sbp.tile([128, B * H], f32, tag=f"xlw{mt}", bufs=2, name=f"xlw{mt}")
                xws = stp.tile([128, B * H], bf16, tag="xws", bufs=2, name="xws")
                for b in range(B):
                    pf = pp.tile([128, 2 * H], f32, tag="pfeat", bufs=2, name="pf")
                    nc.tensor.matmul(pf[0:m, :], xT_cur[(b, mt)][:, 0:m], wt[:, :])
                    nc.scalar.activation(xws[0:m, b * H : (b + 1) * H], pf[0:m, 0:H], Copy)
                    nc.vector.tensor_copy(xlw_new[0:m, b * H : (b + 1) * H], pf[0:m, H : 2 * H])
                nc.gpsimd.dma_start(bounce[moff - roff : moff - roff + m, :], xws[0:m, :])
                xlw_cur[mt] = xlw_new

            def emit_out_feature(mt, bounce, roff):
                m, moff = MT[mt], m_offs[mt]
                mp = (m + 15) // 16 * 16
                xwo = stp.tile([128, 16], bf16, tag="xwo", bufs=2, name="xwo")
                xlwo = stp.tile([128, 128], bf16, tag="xlwo", bufs=2, name="xlwo")
                for b in range(B):
                    pf = pp.tile([128, 2 * F_OUT], f32, tag="pfeat", bufs=2, name="pfo")
                    nc.tensor.matmul(pf[0:m, :], xT_cur[(b, mt)][:, 0:m], w_out_t[:, :])
                    nc.scalar.activation(xwo[0:m, b * F_OUT : (b + 1) * F_OUT], pf[0:m, 0:F_OUT], Copy)
                    nc.scalar.activation(
                        xlwo[0:m, b * F_OUT : (b + 1) * F_OUT], pf[0:m, F_OUT : 2 * F_OUT], Copy
                    )
                nc.gpsimd.dma_start(
                    bounce[moff - roff : moff - roff + m, 0 : B * F_OUT], xwo[0:m, 0 : B * F_OUT]
                )
                tsc = stp.tile([128, 128], bf16, tag="xlwoT_scr", bufs=2, name="tsc")
                nc.scalar.dma_start_transpose(tsc[:, 0:mp], xlwo[0:mp, :])
                nc.scalar.activation(xlwoT_sb[:, moff : moff + m], tsc[0 : B * F_OUT, 0:m], Copy)

            def emit_feats(mts, region, g_next):
                cols = B * H if g_next < 7 else 16
                rw = NA if region == "A" else NBr
                roff = 0 if region == "A" else NA
                bounce = dp.tile([rw, cols], bf16, tag=f"bounce{region}", name=f"bounce{region}")
                for mt in mts:
                    if g_next < 7:
                        emit_feature_h(g_next, mt, bounce, roff)
                    else:
                        emit_out_feature(mt, bounce, roff)
                gath = dp.tile(
                    [PA if region == "A" else PB, cols],
                    bf16,
                    tag=f"gath{region}",
                    addr_space="Shared",
                    name=f"gath{region}",
                )
                emit_ag(bounce, gath)
                return gath

            def emit_epi_nonpe(g, mt, pacc):
                m, moff = MT[mt], m_offs[mt]
                mp = (m + 15) // 16 * 16
                s = sbp.tile([128, B * H], f32, tag="stmp", bufs=2, name="s")
                nc.vector.tensor_tensor(s[0:m, :], pacc[0:m, :], xlw_cur[mt][0:m, :], op=add)
                if g == 0:
                    xr = sbp.tile([128, B * H], f32, tag=f"xres{mt}", bufs=2, name=f"xres{mt}")
                    nc.scalar.activation(xr[0:m, :], s[0:m, :], Relu)
                    xres_cur[mt] = xr
                    yrow = sbp.tile([128, B * H], bf16, tag="yrow", bufs=2, name="yrow")
                    nc.scalar.activation(yrow[0:m, :], xr[0:m, :], Copy)
                elif g % 2 == 1:
                    yrow = sbp.tile([128, B * H], bf16, tag="yrow", bufs=2, name="yrow")
                    nc.scalar.activation(yrow[0:m, :], s[0:m, :], Relu)
                else:
                    h2f = sbp.tile([128, B * H], f32, tag="h2f", bufs=2, name="h2f")
                    nc.scalar.activation(h2f[0:m, :], s[0:m, :], Relu)
                    u = sbp.tile([128, B * H], f32, tag="stmp", bufs=2, name="u")
                    nc.vector.tensor_tensor(u[0:m, :], h2f[0:m, :], xres_cur[mt][0:m, :], op=add)
                    xr = sbp.tile([128, B * H], f32, tag=f"xres{mt}", bufs=2, name=f"xres{mt}")
                    nc.scalar.activation(xr[0:m, :], u[0:m, :], Copy, scale=0.5)
                    xres_cur[mt] = xr
                    yrow = sbp.tile([128, B * H], bf16, tag="yrow", bufs=2, name="yrow")
                    nc.scalar.activation(yrow[0:m, :], xr[0:m, :], Copy)
                for b in range(B):
                    xt = sbp.tile([128, 128], bf16, tag=f"xT_{b}_{mt}", bufs=1, name=f"xT_{b}_{mt}")
                    nc.scalar.dma_start_transpose(xt[:, 0:mp], yrow[0:mp, b * H : (b + 1) * H])
                    xT_cur[(b, mt)] = xt
                if g == 6:
                    nc.sync.dma_start(
                        x_res_out[:, moff : moff + m, :].rearrange("b m h -> m b h"),
                        xres_cur[mt][0:m, :],
                    )

            def emit_kloop_region(grp, paccs, region, gath, first, last):
                c0 = m_offs[grp[0]]
                c1 = m_offs[grp[-1]] + MT[grp[-1]]
                gw = c1 - c0
                koff = 0 if region == "A" else KA
                batches = bA if region == "A" else bB
                for bi, (k0, nk, kp) in enumerate(batches):
                    xwk = stp.tile([128, 4, B * H], bf16, tag="xwk", bufs=3, name="xwk")
                    src = gath[k0 * 128 : k0 * 128 + (nk - 1) * 128 + kp, :]
                    nc.sync.dma_start(
                        xwk[0:kp, 0:nk, :], src.rearrange("(kk p) c -> p kk c", kk=nk)
                    )
                    at = stp.tile([128, 4, MAXW], bf16, tag="at", bufs=3, name="at")
                    nc.sync.dma_start(
                        at[0:kp, 0:nk, 0:gw],
                        adjT[koff + k0 : koff + k0 + nk, 0:kp, c0:c1].rearrange("kk p c -> p kk c"),
                    )
                    for kk in range(nk):
                        g_first = first and bi == 0 and kk == 0
                        g_last = last and bi == len(batches) - 1 and kk == nk - 1
                        for i, mt in enumerate(grp):
                            m = MT[mt]
                            o0 = m_offs[mt] - c0
                            nc.tensor.matmul(
                                paccs[i][0:m, :],
                                at[0:kp, kk, o0 : o0 + m],
                                xwk[0:kp, kk, :],
                                start=g_first,
                                stop=g_last,
                            )

            def emit_l0(region, mts):
                rw = NA if region == "A" else NBr
                roff = 0 if region == "A" else NA
                bounce = dp.tile([rw, B * H], bf16, tag=f"bounce{region}", name=f"bounce{region}")
                for b in range(B):
                    x0t = stp.tile([128, KF, MAXW], bf16, tag="x0t", bufs=2, name="x0t")
                    nc.sync.dma_start(
                        x0t[:, :, 0:rw],
                        x0T[b, :, :, roff : roff + rw].rearrange("kf p c -> p kf c"),
                    )
                    for mt in mts:
                        m, moff = MT[mt], m_offs[mt]
                        if b == 0:
                            xlw_cur[mt] = sbp.tile(
                                [128, B * H], f32, tag=f"xlw{mt}", bufs=2, name=f"xlw{mt}"
                            )
                        pf = pp.tile([128, 2 * H], f32, tag="pfeat", bufs=2, name="pf0")
                        for kf in range(KF):
                            nc.tensor.matmul(
                                pf[0:m, :],
                                x0t[:, kf, moff - roff : moff - roff + m],
                                w_in_t[kf][:, :],
                                start=(kf == 0),
                                stop=(kf == KF - 1),
                            )
                        xws = stp.tile([128, H], bf16, tag="xws0", bufs=2, name="xws0")
                        nc.scalar.activation(xws[0:m, :], pf[0:m, 0:H], Copy)
                        nc.gpsimd.dma_start(
                            bounce[moff - roff : moff - roff + m, b * H : (b + 1) * H], xws[0:m, :]
                        )
                        nc.vector.tensor_copy(xlw_cur[mt][0:m, b * H : (b + 1) * H], pf[0:m, H : 2 * H])
                gath = dp.tile(
                    [PA if region == "A" else PB, B * H],
                    bf16,
                    tag=f"gath{region}",
                    addr_space="Shared",
                    name=f"gath{region}",
                )
                emit_ag(bounce, gath)
                return gath

            def emit_out_adj(gA7, gB7):
                banks = pk_banks(len(out_chunks))
                pouts = [
                    pp.tile([B * F_OUT, 512], f32, tag=t, bufs=1, name=f"pout{t}") for t in banks
                ]
                for region, gath in (("A", gA7), ("B", gB7)):
                    koff = 0 if region == "A" else KA
                    batches = bA if region == "A" else bB
                    for bi, (k0, nk, kp) in enumerate(batches):
                        xwk = stp.tile([128, 4, 16], bf16, tag="xwok", bufs=3, name="xwok")
                        src = gath[k0 * 128 : k0 * 128 + (nk - 1) * 128 + kp, :]
                        nc.sync.dma_start(
                            xwk[0:kp, 0:nk, :], src.rearrange("(kk p) c -> p kk c", kk=nk)
                        )
                        at = stp.tile([128, 4, NL], bf16, tag="ato", bufs=2, name="ato")
                        nc.sync.dma_start(
                            at[0:kp, 0:nk, :],
                            adjT[koff + k0 : koff + k0 + nk, 0:kp, :].rearrange("kk p c -> p kk c"),
                        )
                        for kk in range(nk):
                            g_first = region == "A" and bi == 0 and kk == 0
                            g_last = region == "B" and bi == len(batches) - 1 and kk == nk - 1
                            for ci, (c0, c1) in enumerate(out_chunks):
                                nc.tensor.matmul(
                                    pouts[ci][:, 0 : c1 - c0],
                                    xwk[0:kp, kk, 0 : B * F_OUT],
                                    at[0:kp, kk, c0:c1],
                                    start=g_first,
                                    stop=g_last,
                                )
                youtT = sbp.tile([B * F_OUT, NL], f32, tag="youtT", name="youtT")
                for ci, (c0, c1) in enumerate(out_chunks):
                    nc.vector.tensor_tensor(
                        youtT[:, c0:c1], pouts[ci][:, 0 : c1 - c0], xlwoT_sb[:, c0:c1], op=add
                    )
                nc.sync.dma_start(x_out_t[:, :], youtT[:, :])

            # ---- pipeline ----
            gA = emit_l0("A", A_mts)
            gB = None
            for g in range(7):
                if g == 0:
                    gB = emit_l0("B", B_mts)
                else:
                    gB = emit_feats(B_mts, "B", g)
                banksA = pk_banks(len(A_mts))
                paccsA = [pp.tile([128, B * H], f32, tag=t, bufs=1, name=t) for t in banksA]
                emit_kloop_region(A_mts, paccsA, "A", gA, first=True, last=False)
                emit_kloop_region(A_mts, paccsA, "B", gB, first=False, last=True)
                for i, mt in enumerate(A_mts):
                    emit_epi_nonpe(g, mt, paccsA[i])
                gA_next = emit_feats(A_mts, "A", g + 1)
                banksB = pk_banks(len(B_mts))
                paccsB = [pp.tile([128, B * H], f32, tag=t, bufs=1, name=t) for t in banksB]
                emit_kloop_region(B_mts, paccsB, "A", gA, first=True, last=False)
                emit_kloop_region(B_mts, paccsB, "B", gB, first=False, last=True)
                for i, mt in enumerate(B_mts):
                    emit_epi_nonpe(g, mt, paccsB[i])
                gA = gA_next
            gB = emit_feats(B_mts, "B", 7)
            emit_out_adj(gA, gB)

    nc.finalize()
    return nc


def _host_prep(shape_verts, adj, in_w, in_lw, res_w1, res_lw1, res_w2, res_lw2, out_w, out_lw, N, P, F_in):
    NL = N // P
    MT = _tiles(NL)
    NMT = len(MT)
    NMT_A = (NMT + 1) // 2
    NA = sum(MT[:NMT_A])
    NBr = NL - NA
    PA, PB = P * NA, P * NBr
    KA = PA // 128
    KBt = (PB + 127) // 128
    NKT = KA + KBt
    KF = (F_in + 127) // 128
    FP = KF * 128

    idxA = np.arange(PA)
    idxB = np.arange(PB)
    jperm = np.concatenate(
        [
            (idxA // NA) * NL + (idxA % NA),
            (idxB // NBr) * NL + NA + (idxB % NBr),
        ]
    )
    adjT_full = np.zeros((NKT * 128, N), dtype=BF)
    adjT_full[:N, :] = adj.T.astype(BF)[jperm, :]
    adjT_full = adjT_full.reshape(NKT, 128, N)

    w_cat = np.zeros((FP, 2 * H), np.float32)
    w_cat[:F_in, :H] = in_w
    w_cat[:F_in, H:] = in_lw
    w_in_h = w_cat.reshape(KF, 128, 2 * H).astype(BF)

    w_res_h = np.zeros((2 * NBLK, H, 2 * H), np.float32)
    for i in range(NBLK):
        w_res_h[2 * i, :, :H] = res_w1[i]
        w_res_h[2 * i, :, H:] = res_lw1[i]
        w_res_h[2 * i + 1, :, :H] = res_w2[i]
        w_res_h[2 * i + 1, :, H:] = res_lw2[i]
    w_res_h = w_res_h.astype(BF)

    w_out_h = np.concatenate([out_w, out_lw], axis=1).astype(BF)

    in_maps = []
    for c in range(P):
        rows = slice(c * NL, (c + 1) * NL)
        svc = shape_verts[:, rows, :].transpose(0, 2, 1).astype(BF)  # [B, F_in, NL]
        x0T_c = np.zeros((B, KF * 128, NL), dtype=BF)
        x0T_c[:, :F_in, :] = svc
        x0T_c = x0T_c.reshape(B, KF, 128, NL)
        adjT_c = np.ascontiguousarray(adjT_full[:, :, rows])
        in_maps.append(
            {
                "x0T": x0T_c,
                "adjT": adjT_c,
                "w_in": w_in_h,
                "w_res": w_res_h,
                "w_out": w_out_h,
            }
        )
    return in_maps


def run(inputs, N, P, F_in, trace=False):
    from concourse import bass_utils

    nc = build_nc(N, P, F_in)
    in_maps = _host_prep(
        inputs["shape_verts"], inputs["adj"],
        inputs["in_w"], inputs["in_lw"],
        inputs["res_w1"], inputs["res_lw1"],
        inputs["res_w2"], inputs["res_lw2"],
        inputs["out_w"], inputs["out_lw"],
        N, P, F_in,
    )
    res = bass_utils.run_bass_kernel_spmd(nc, in_maps, list(range(P)), trace=trace)
    NL = N // P
    x_full = np.empty((B, N, H), np.float32)
    x_out = np.empty((B, N, F_OUT), np.float32)
    for c in range(P):
        x_full[:, c * NL : (c + 1) * NL, :] = res.results[c]["x_res_out"]
        yt = np.asarray(res.results[c]["x_out_t"]).reshape(B, F_OUT, NL)
        x_out[:, c * NL : (c + 1) * NL, :] = yt.transpose(0, 2, 1)
    return (x_out, x_full), res


def kernel(**inputs):
    (x_out, x_full), _ = run(inputs, N_FULL, P_CORES, F_IN)
    return (x_out, x_full)


# revision 17
# speedup vs baseline: 1.3811x; 1.3811x over previous
import sys

import numpy as np

if "/opt/trn_rl_repo" not in sys.path:
    sys.path.insert(0, "/opt/trn_rl_repo")

import ml_dtypes

BF = ml_dtypes.bfloat16
F8 = ml_dtypes.float8_e4m3
ADJ_SCALE = 1024.0
DESCALE = 1.0 / ADJ_SCALE

B = 4
H = 128
F_OUT = 3
NBLK = 3
N_FULL = 10000
P_CORES = 8
F_IN = 963


def _tiles(total, t=128):
    out = []
    while total > 0:
        out.append(min(t, total))
        total -= t
    return out


def _kbatches(rows, maxb=4):
    full = rows // 128
    rem = rows % 128
    bs = []
    k = 0
    while k < full:
        n = min(maxb, full - k)
        bs.append((k, n, 128))
        k += n
    if rem:
        bs.append((full, 1, rem))
    return bs


def build_nc(N, P, F_in):
    """SPMD Bass program (identical on all cores).

    adj rows are sharded (NL per core). Row tiles are split into two halves
    (A = first 5 m-tiles, B = rest); each half's transformed features are
    all-gathered separately so the collective for one half hides under the
    adjacency matmuls of the other. adjT is host-permuted so the gathered
    [A-of-all-cores; B-of-all-cores] row order matches its k dimension.
    """
    from concourse import bacc, tile, mybir

    f32 = mybir.dt.float32
    bf16 = mybir.dt.bfloat16
    fp8 = mybir.dt.float8e4
    DR = mybir.MatmulPerfMode.DoubleRow
    Relu = mybir.ActivationFunctionType.Relu
    Copy = mybir.ActivationFunctionType.Copy
    add = mybir.AluOpType.add

    NL = N // P
    MT = _tiles(NL)
    NMT = len(MT)
    m_offs = [sum(MT[:i]) for i in range(NMT)]
    assert NMT >= 2
    NMT_A = (NMT + 1) // 2
    A_mts = list(range(NMT_A))
    B_mts = list(range(NMT_A, NMT))
    NA = sum(MT[:NMT_A])
    NBr = NL - NA
    PA, PB = P * NA, P * NBr
    assert PA % 128 == 0
    KA = PA // 128
    KBt = (PB + 127) // 128
    NKT = KA + KBt
    bA = _kbatches(PA)
    bB = _kbatches(PB)
    KF = (F_in + 127) // 128
    MAXW = max(NA, NBr)
    out_chunks = []
    c = 0
    while c < NL:
        out_chunks.append((c, min(c + 512, NL)))
        c += 512

    nc = bacc.Bacc(trn_type="TRN2", target_bir_lowering=False, num_devices=P)

    x0T = nc.dram_tensor("x0T", [B, KF, 128, NL], bf16, kind="ExternalInput")
    adjT = nc.dram_tensor("adjT", [NKT, 128, NL], fp8, kind="ExternalInput")
    w_in = nc.dram_tensor("w_in", [KF, 128, 2 * H], bf16, kind="ExternalInput")
    w_res = nc.dram_tensor("w_res", [2 * NBLK, 128, 2 * H], bf16, kind="ExternalInput")
    w_out = nc.dram_tensor("w_out", [128, 2 * F_OUT], bf16, kind="ExternalInput")
    x_res_out = nc.dram_tensor("x_res_out", [B, NL, H], f32, kind="ExternalOutput")
    x_out_t = nc.dram_tensor("x_out_t", [B * F_OUT, NL], f32, kind="ExternalOutput")

    replica_groups = [list(range(P))]

    with tile.TileContext(nc) as tc:
        with (
            tc.tile_pool(name="wp", bufs=1) as wp,
            tc.tile_pool(name="sbp", bufs=1) as sbp,
            tc.tile_pool(name="stp", bufs=1) as stp,
            tc.tile_pool(name="pp", bufs=1, space="PSUM") as pp,
            tc.tile_pool(name="dp", bufs=2, space="DRAM") as dp,
        ):
            w_in_t = []
            for kf in range(KF):
                wt = wp.tile([128, 2 * H], bf16, tag=f"w_in{kf}", name=f"w_in{kf}")
                nc.sync.dma_start(wt[:, :], w_in[kf, :, :])
                w_in_t.append(wt)
            w_res_t = []
            for i in range(2 * NBLK):
                wt = wp.tile([128, 2 * H], bf16, tag=f"w_res{i}", name=f"w_res{i}")
                nc.sync.dma_start(wt[:, :], w_res[i, :, :])
                w_res_t.append(wt)
            w_out_t = wp.tile([128, 2 * F_OUT], bf16, tag="w_out", name="w_out")
            nc.sync.dma_start(w_out_t[:, :], w_out[:, :])
            xlwoT_sb = sbp.tile([F_OUT * B, NL], f32, tag="xlwoT", name="xlwoT_sb")

            xT_cur = {}
            xlw_cur = {}
            xres_cur = {}
            st = {"pk": 0}

            def pk_banks(n):
                c = st["pk"]
                st["pk"] = c + n
                return [f"pk{(c + i) % 6}" for i in range(n)]

            def emit_ag(bounce, gath):
                nc.gpsimd.collective_compute(
                    "AllGather",
                    mybir.AluOpType.bypass,
                    replica_groups=replica_groups,
                    ins=[bounce.opt()],
                    outs=[gath.opt()],
                )

            def emit_feature_h(g_next, mt, bounce, roff):
                m, moff = MT[mt], m_offs[mt]
                wt = w_res_t[g_next - 1]
                xlw_new = sbp.tile([128, B * H], f32, tag=f"xlw{mt}", bufs=2, name=f"xlw{mt}")
                xws = stp.tile([128, B * H], fp8, tag="xws", bufs=2, name="xws")
                for b in range(B):
                    pf = pp.tile([128, 2 * H], f32, tag="pfeat", bufs=2, name="pf")
                    nc.tensor.matmul(pf[0:m, :], xT_cur[(b, mt)][:, 0:m], wt[:, :])
                    nc.scalar.activation(xws[0:m, b * H : (b + 1) * H], pf[0:m, 0:H], Copy)
                    nc.vector.tensor_copy(xlw_new[0:m, b * H : (b + 1) * H], pf[0:m, H : 2 * H])
                nc.gpsimd.dma_start(bounce[moff - roff : moff - roff + m, :], xws[0:m, :])
                xlw_cur[mt] = xlw_new

            def emit_out_feature(mt, bounce, roff):
                m, moff = MT[mt], m_offs[mt]
                mp = (m + 15) // 16 * 16
                xwo = stp.tile([128, 16], fp8, tag="xwo", bufs=2, name="xwo")
                xlwo = stp.tile([128, 128], bf16, tag="xlwo", bufs=2, name="xlwo")
                for b in range(B):
                    pf = pp.tile([128, 2 * F_OUT], f32, tag="pfeat", bufs=2, name="pfo")
                    nc.tensor.matmul(pf[0:m, :], xT_cur[(b, mt)][:, 0:m], w_out_t[:, :])
                    nc.scalar.activation(xwo[0:m, b * F_OUT : (b + 1) * F_OUT], pf[0:m, 0:F_OUT], Copy)
                    nc.scalar.activation(
                        xlwo[0:m, b * F_OUT : (b + 1) * F_OUT], pf[0:m, F_OUT : 2 * F_OUT], Copy
                    )
                nc.gpsimd.dma_start(
                    bounce[moff - roff : moff - roff + m, 0 : B * F_OUT], xwo[0:m, 0 : B * F_OUT]
                )
                tsc = stp.tile([128, 128], bf16, tag="xlwoT_scr", bufs=2, name="tsc")
                nc.scalar.dma_start_transpose(tsc[:, 0:mp], xlwo[0:mp, :])
                nc.scalar.activation(xlwoT_sb[:, moff : moff + m], tsc[0 : B * F_OUT, 0:m], Copy)

            def emit_feats(mts, region, g_next):
                cols = B * H if g_next < 7 else 16
                rw = NA if region == "A" else NBr
                roff = 0 if region == "A" else NA
                bounce = dp.tile([rw, cols], fp8, tag=f"bounce{region}", name=f"bounce{region}")
                for mt in mts:
                    if g_next < 7:
                        emit_feature_h(g_next, mt, bounce, roff)
                    else:
                        emit_out_feature(mt, bounce, roff)
                gath = dp.tile(
                    [PA if region == "A" else PB, cols],
                    fp8,
                    tag=f"gath{region}",
                    addr_space="Shared",
                    name=f"gath{region}",
                )
                emit_ag(bounce, gath)
                return gath

            def emit_epi_nonpe(g, mt, pacc):
                m, moff = MT[mt], m_offs[mt]
                mp = (m + 15) // 16 * 16
                s = sbp.tile([128, B * H], f32, tag="stmp", bufs=2, name="s")
                nc.vector.tensor_tensor(s[0:m, :], pacc[0:m, :], xlw_cur[mt][0:m, :], op=add)
                if g == 0:
                    xr = sbp.tile([128, B * H], f32, tag=f"xres{mt}", bufs=2, name=f"xres{mt}")
                    nc.scalar.activation(xr[0:m, :], s[0:m, :], Relu, scale=DESCALE)
                    xres_cur[mt] = xr
                    yrow = sbp.tile([128, B * H], bf16, tag="yrow", bufs=2, name="yrow")
                    nc.scalar.activation(yrow[0:m, :], xr[0:m, :], Copy)
                elif g % 2 == 1:
                    yrow = sbp.tile([128, B * H], bf16, tag="yrow", bufs=2, name="yrow")
                    nc.scalar.activation(yrow[0:m, :], s[0:m, :], Relu, scale=DESCALE)
                else:
                    h2f = sbp.tile([128, B * H], f32, tag="h2f", bufs=2, name="h2f")
                    nc.scalar.activation(h2f[0:m, :], s[0:m, :], Relu, scale=DESCALE)
                    u = sbp.tile([128, B * H], f32, tag="stmp", bufs=2, name="u")
                    nc.vector.tensor_tensor(u[0:m, :], h2f[0:m, :], xres_cur[mt][0:m, :], op=add)
                    xr = sbp.tile([128, B * H], f32, tag=f"xres{mt}", bufs=2, name=f"xres{mt}")
                    nc.scalar.activation(xr[0:m, :], u[0:m, :], Copy, scale=0.5)
                    xres_cur[mt] = xr
                    yrow = sbp.tile([128, B * H], bf16, tag="yrow", bufs=2, name="yrow")
                    nc.scalar.activation(yrow[0:m, :], xr[0:m, :], Copy)
                for b in range(B):
                    xt = sbp.tile([128, 128], bf16, tag=f"xT_{b}_{mt}", bufs=1, name=f"xT_{b}_{mt}")
                    nc.scalar.dma_start_transpose(xt[:, 0:mp], yrow[0:mp, b * H : (b + 1) * H])
                    xT_cur[(b, mt)] = xt
                if g == 6:
                    nc.sync.dma_start(
                        x_res_out[:, moff : moff + m, :].rearrange("b m h -> m b h"),
                        xres_cur[mt][0:m, :],
                    )

            def emit_kloop_region(grp, paccs, region, gath, first, last):
                c0 = m_offs[grp[0]]
                c1 = m_offs[grp[-1]] + MT[grp[-1]]
                gw = c1 - c0
                koff = 0 if region == "A" else KA
                batches = bA if region == "A" else bB
                for bi, (k0, nk, kp) in enumerate(batches):
                    xwk = stp.tile([128, 4, B * H], fp8, tag="xwk", bufs=3, name="xwk")
                    src = gath[k0 * 128 : k0 * 128 + (nk - 1) * 128 + kp, :]
                    nc.sync.dma_start(
                        xwk[0:kp, 0:nk, :], src.rearrange("(kk p) c -> p kk c", kk=nk)
                    )
                    at = stp.tile([128, 4, MAXW], fp8, tag="at", bufs=3, name="at")
                    nc.sync.dma_start(
                        at[0:kp, 0:nk, 0:gw],
                        adjT[koff + k0 : koff + k0 + nk, 0:kp, c0:c1].rearrange("kk p c -> p kk c"),
                    )
                    kk = 0
                    while kk < nk:
                        dbl = kk + 1 < nk and kp == 128
                        g_first = first and bi == 0 and kk == 0
                        g_last = last and bi == len(batches) - 1 and kk + (2 if dbl else 1) == nk
                        for i, mt in enumerate(grp):
                            m = MT[mt]
                            o0 = m_offs[mt] - c0
                            if dbl:
                                nc.tensor.matmul(
                                    paccs[i][0:m, :],
                                    at[0:kp, kk : kk + 2, o0 : o0 + m],
                                    xwk[0:kp, kk : kk + 2, :],
                                    start=g_first,
                                    stop=g_last,
                                    perf_mode=DR,
                                )
                            else:
                                nc.tensor.matmul(
                                    paccs[i][0:m, :],
                                    at[0:kp, kk, o0 : o0 + m],
                                    xwk[0:kp, kk, :],
                                    start=g_first,
                                    stop=g_last,
                                )
                        kk += 2 if dbl else 1

            def emit_l0(region, mts):
                rw = NA if region == "A" else NBr
                roff = 0 if region == "A" else NA
                bounce = dp.tile([rw, B * H], fp8, tag=f"bounce{region}", name=f"bounce{region}")
                for b in range(B):
                    x0t = stp.tile([128, KF, MAXW], bf16, tag="x0t", bufs=2, name="x0t")
                    nc.sync.dma_start(
                        x0t[:, :, 0:rw],
                        x0T[b, :, :, roff : roff + rw].rearrange("kf p c -> p kf c"),
                    )
                    for mt in mts:
                        m, moff = MT[mt], m_offs[mt]
                        if b == 0:
                            xlw_cur[mt] = sbp.tile(
                                [128, B * H], f32, tag=f"xlw{mt}", bufs=2, name=f"xlw{mt}"
                            )
                        pf = pp.tile([128, 2 * H], f32, tag="pfeat", bufs=2, name="pf0")
                        for kf in range(KF):
                            nc.tensor.matmul(
                                pf[0:m, :],
                                x0t[:, kf, moff - roff : moff - roff + m],
                                w_in_t[kf][:, :],
                                start=(kf == 0),
                                stop=(kf == KF - 1),
                            )
                        xws = stp.tile([128, H], fp8, tag="xws0", bufs=2, name="xws0")
                        nc.scalar.activation(xws[0:m, :], pf[0:m, 0:H], Copy)
                        nc.gpsimd.dma_start(
                            bounce[moff - roff : moff - roff + m, b * H : (b + 1) * H], xws[0:m, :]
                        )
                        nc.vector.tensor_copy(xlw_cur[mt][0:m, b * H : (b + 1) * H], pf[0:m, H : 2 * H])
                gath = dp.tile(
                    [PA if region == "A" else PB, B * H],
                    fp8,
                    tag=f"gath{region}",
                    addr_space="Shared",
                    name=f"gath{region}",
                )
                emit_ag(bounce, gath)
                return gath

            def emit_out_adj(gA7, gB7):
                banks = pk_banks(len(out_chunks))
                pouts = [
                    pp.tile([B * F_OUT, 512], f32, tag=t, bufs=1, name=f"pout{t}") for t in banks
                ]
                for region, gath in (("A", gA7), ("B", gB7)):
                    koff = 0 if region == "A" else KA
                    batches = bA if region == "A" else bB
                    for bi, (k0, nk, kp) in enumerate(batches):
                        xwk = stp.tile([128, 4, 16], fp8, tag="xwok", bufs=3, name="xwok")
                        src = gath[k0 * 128 : k0 * 128 + (nk - 1) * 128 + kp, :]
                        nc.sync.dma_start(
                            xwk[0:kp, 0:nk, :], src.rearrange("(kk p) c -> p kk c", kk=nk)
                        )
                        at = stp.tile([128, 4, NL], fp8, tag="ato", bufs=2, name="ato")
                        nc.sync.dma_start(
                            at[0:kp, 0:nk, :],
                            adjT[koff + k0 : koff + k0 + nk, 0:kp, :].rearrange("kk p c -> p kk c"),
                        )
                        kk = 0
                        while kk < nk:
                            dbl = kk + 1 < nk and kp == 128
                            g_first = region == "A" and bi == 0 and kk == 0
                            g_last = (
                                region == "B"
                                and bi == len(batches) - 1
                                and kk + (2 if dbl else 1) == nk
                            )
                            for ci, (c0, c1) in enumerate(out_chunks):
                                if dbl:
                                    nc.tensor.matmul(
                                        pouts[ci][:, 0 : c1 - c0],
                                        xwk[0:kp, kk : kk + 2, 0 : B * F_OUT],
                                        at[0:kp, kk : kk + 2, c0:c1],
                                        start=g_first,
                                        stop=g_last,
                                        perf_mode=DR,
                                    )
                                else:
                                    nc.tensor.matmul(
                                        pouts[ci][:, 0 : c1 - c0],
                                        xwk[0:kp, kk, 0 : B * F_OUT],
                                        at[0:kp, kk, c0:c1],
                                        start=g_first,
                                        stop=g_last,
                                    )
                            kk += 2 if dbl else 1
                youtT = sbp.tile([B * F_OUT, NL], f32, tag="youtT", name="youtT")
                yscl = sbp.tile([B * F_OUT, NL], f32, tag="yscl", name="yscl")
                for ci, (c0, c1) in enumerate(out_chunks):
                    nc.vector.tensor_tensor(
                        youtT[:, c0:c1], pouts[ci][:, 0 : c1 - c0], xlwoT_sb[:, c0:c1], op=add
                    )
                nc.scalar.activation(yscl[:, :], youtT[:, :], Copy, scale=DESCALE)
                nc.sync.dma_start(x_out_t[:, :], yscl[:, :])

            # ---- pipeline ----
            gA = emit_l0("A", A_mts)
            gB = None
            for g in range(7):
                if g == 0:
                    gB = emit_l0("B", B_mts)
                else:
                    gB = emit_feats(B_mts, "B", g)
                banksA = pk_banks(len(A_mts))
                paccsA = [pp.tile([128, B * H], f32, tag=t, bufs=1, name=t) for t in banksA]
                emit_kloop_region(A_mts, paccsA, "A", gA, first=True, last=False)
                emit_kloop_region(A_mts, paccsA, "B", gB, first=False, last=True)
                for i, mt in enumerate(A_mts):
                    emit_epi_nonpe(g, mt, paccsA[i])
                gA_next = emit_feats(A_mts, "A", g + 1)
                banksB = pk_banks(len(B_mts))
                paccsB = [pp.tile([128, B * H], f32, tag=t, bufs=1, name=t) for t in banksB]
                emit_kloop_region(B_mts, paccsB, "A", gA, first=True, last=False)
                emit_kloop_region(B_mts, paccsB, "B", gB, first=False, last=True)
                for i, mt in enumerate(B_mts):
                    emit_epi_nonpe(g, mt, paccsB[i])
                gA = gA_next
            gB = emit_feats(B_mts, "B", 7)
            emit_out_adj(gA, gB)

    nc.finalize()
    return nc


def _host_prep(shape_verts, adj, in_w, in_lw, res_w1, res_lw1, res_w2, res_lw2, out_w, out_lw, N, P, F_in):
    NL = N // P
    MT = _tiles(NL)
    NMT = len(MT)
    NMT_A = (NMT + 1) // 2
    NA = sum(MT[:NMT_A])
    NBr = NL - NA
    PA, PB = P * NA, P * NBr
    KA = PA // 128
    KBt = (PB + 127) // 128
    NKT = KA + KBt
    KF = (F_in + 127) // 128
    FP = KF * 128

    idxA = np.arange(PA)
    idxB = np.arange(PB)
    jperm = np.concatenate(
        [
            (idxA // NA) * NL + (idxA % NA),
            (idxB // NBr) * NL + NA + (idxB % NBr),
        ]
    )
    adjT_full = np.zeros((NKT * 128, N), dtype=F8)
    adjT_full[:N, :] = (adj.T.astype(np.float32) * ADJ_SCALE).astype(F8)[jperm, :]
    adjT_full = adjT_full.reshape(NKT, 128, N)

    w_cat = np.zeros((FP, 2 * H), np.float32)
    w_cat[:F_in, :H] = in_w
    w_cat[:F_in, H:] = in_lw * ADJ_SCALE
    w_in_h = w_cat.reshape(KF, 128, 2 * H).astype(BF)

    w_res_h = np.zeros((2 * NBLK, H, 2 * H), np.float32)
    for i in range(NBLK):
        w_res_h[2 * i, :, :H] = res_w1[i]
        w_res_h[2 * i, :, H:] = res_lw1[i] * ADJ_SCALE
        w_res_h[2 * i + 1, :, :H] = res_w2[i]
        w_res_h[2 * i + 1, :, H:] = res_lw2[i] * ADJ_SCALE
    w_res_h = w_res_h.astype(BF)

    w_out_h = np.concatenate([out_w, out_lw * ADJ_SCALE], axis=1).astype(BF)

    in_maps = []
    for c in range(P):
        rows = slice(c * NL, (c + 1) * NL)
        svc = shape_verts[:, rows, :].transpose(0, 2, 1).astype(BF)  # [B, F_in, NL]
        x0T_c = np.zeros((B, KF * 128, NL), dtype=BF)
        x0T_c[:, :F_in, :] = svc
        x0T_c = x0T_c.reshape(B, KF, 128, NL)
        adjT_c = np.ascontiguousarray(adjT_full[:, :, rows])
        in_maps.append(
            {
                "x0T": x0T_c,
                "adjT": adjT_c,
                "w_in": w_in_h,
                "w_res": w_res_h,
                "w_out": w_out_h,
            }
        )
    return in_maps


def run(inputs, N, P, F_in, trace=False):
    from concourse import bass_utils

    nc = build_nc(N, P, F_in)
    in_maps = _host_prep(
        inputs["shape_verts"], inputs["adj"],
        inputs["in_w"], inputs["in_lw"],
        inputs["res_w1"], inputs["res_lw1"],
        inputs["res_w2"], inputs["res_lw2"],
        inputs["out_w"], inputs["out_lw"],
        N, P, F_in,
    )
    res = bass_utils.run_bass_kernel_spmd(nc, in_maps, list(range(P)), trace=trace)
    NL = N // P
    x_full = np.empty((B, N, H), np.float32)
    x_out = np.empty((B, N, F_OUT), np.float32)
    for c in range(P):
        x_full[:, c * NL : (c + 1) * NL, :] = res.results[c]["x_res_out"]
        yt = np.asarray(res.results[c]["x_out_t"]).reshape(B, F_OUT, NL)
        x_out[:, c * NL : (c + 1) * NL, :] = yt.transpose(0, 2, 1)
    return (x_out, x_full), res


def kernel(**inputs):
    (x_out, x_full), _ = run(inputs, N_FULL, P_CORES, F_IN)
    return (x_out, x_full)
